# revision 2
# baseline (speedup 1.0000x reference)
"""DecoderRNN (GRU + embedding + vocab projection) Bass kernel, v2.

Gates-major recurrence: gate rows live on PSUM partitions (24 chunks of
128 across r/z/n), batch=64 is the streamed free dim. Each matmul runs
the full 128-wide PE array (vs. 64-wide in the batch-major layout), so
the recurrence costs 18.4k PE cycles/step instead of 36.9k — and h'
comes out of the gate chain already in the [H-on-partitions, batch]
layout the next step's matmuls (and the output projection's lhsT) need,
so the per-step transpose matmul disappears entirely.

  toks = [2, x[0..S-2]]                  (teacher forcing, S=64, B=64)
  e_s  = relu(emb[toks_s])               (E=512, padding row 0 = 0)
  r/z  = sig(W e + U h + b), n = tanh(gin + r*ghn),  h' = n + z*(h-n)
  logits_s = h_s @ Wout^T + b_out        (V=32000)

Distribution: recurrence redundant on all 8 cores (SPMD single-program;
per-step cross-core gathers cost more than they save), output projection
vocab-split 8 ways (4000 cols/core), its matmuls interleaved into the
per-step gate-chain windows to keep the PE busy.

Layouts (gates-major):
  hs tile [128, 8, 128] fp16 per step-pair g: hs[p, kc, 64*half+b]
    = h_{2g+half}[b, 128*kc+p] — rhs for the recurrence (stream batch),
    lhsT for the output projection (128-token stationary).
  psum ps_rz [128, 16, 64]: chunk c = gate r rows 128c..', chunk 8+c = z.
  ps_hn/ps_gin [128, 8, 64]: n-gate h-part / e-part (kept separate).
  w_* [128, kc, gc, 128] fp16: w[p, kc, gc, m] = W[128*gc+m, 128*kc+p].
Matmuls fp16 (fp32 PSUM accumulate); gate arithmetic fp32 with biases
fused into ACT sigmoid / Pool tensor_scalar; h carried in fp16 only.
"""

import sys

sys.path.insert(0, "/opt/trn_rl_repo")

import numpy as np

import concourse.bass as bass
import concourse.bacc as bacc
import concourse.mybir as mybir
import concourse.tile as tile
from concourse.bass_utils import run_bass_kernel_spmd

FP16 = mybir.dt.float16
F32 = mybir.dt.float32
I32 = mybir.dt.int32
SIG = mybir.ActivationFunctionType.Sigmoid
TANH = mybir.ActivationFunctionType.Tanh
ADD = mybir.AluOpType.add
SUB = mybir.AluOpType.subtract
MULT = mybir.AluOpType.mult

S, B, H, E, V = 64, 64, 1024, 512, 32000
NCORES = 8
VC = V // NCORES          # 4000 vocab cols per core
SB = S * B                # 4096
KH = H // 128             # 8 h k-chunks
KE = E // 128             # 4 e k-chunks
NN = 8                    # output n-chunks per core
NW = VC // NN             # 500 cols per n-chunk
NT = SB // 128            # 32 output row tiles (= step pairs)

_CACHE = {}


def _build(n_steps=S, with_jobs=True):
    key = ("nc2", n_steps, with_jobs)
    if key in _CACHE:
        return _CACHE[key]

    nc = bacc.Bacc("TRN2", target_bir_lowering=False, debug=False)

    def din(name, shape, dt):
        return nc.dram_tensor(name, shape, dt, kind="ExternalInput").ap()

    emb_d = din("emb_t", [V, E], FP16)
    idx_d = din("idx", [128, NT], I32)
    w_rz_d = din("w_rz", [16, 128, KH, 128], FP16)
    w_hn_d = din("w_hn", [8, 128, KH, 128], FP16)
    w_irz_d = din("w_irz", [16, 128, KE, 128], FP16)
    w_in_d = din("w_in", [8, 128, KE, 128], FP16)
    b_r_d = din("b_r", [128, 8], F32)
    b_z_d = din("b_z", [128, 8], F32)
    b_hn_d = din("b_hn", [128, 8], F32)
    b_in_d = din("b_in", [128, 8], F32)
    h0_d = din("h0", [128, 8, 64], FP16)
    w_outT_d = din("w_outT", [NN, 128, KH, NW], FP16)
    b_out_d = din("b_out_bc", [128, VC], FP16)
    out_d = nc.dram_tensor("out", [SB, VC], F32, kind="ExternalOutput").ap()

    with tile.TileContext(nc) as tc:
        with tc.tile_pool(name="const", bufs=1) as pc, \
             tc.tile_pool(name="roll", bufs=1) as pr, \
             tc.tile_pool(name="psum", bufs=1, space="PSUM") as pp:

            # ---- constants in SBUF, DMA'd in consumption order so step 0
            # starts after ~1 MB instead of the full 18 MB of weights.
            b_r = pc.tile([128, 8], F32, name="b_r")
            b_z = pc.tile([128, 8], F32, name="b_z")
            b_hn = pc.tile([128, 8], F32, name="b_hn")
            b_in = pc.tile([128, 8], F32, name="b_in")
            h0 = pc.tile([128, 8, 64], FP16, name="h0")
            b_out = pc.tile([128, VC], FP16, name="b_out")
            idx = pc.tile([128, NT], I32, name="idx")
            for t, d in [(idx, idx_d), (h0, h0_d), (b_r, b_r_d), (b_z, b_z_d),
                         (b_hn, b_hn_d), (b_in, b_in_d)]:
                nc.sync.dma_start(out=t[:], in_=d[:])

            # ---- embedding gather pipeline (tile g = tokens of steps 2g, 2g+1)
            # primed BEFORE the weight DMAs: the transposes ride the SP HWDGE
            # queue, and behind 17.6 MB of weights the first eT would otherwise
            # arrive at ~66us instead of ~5us.
            def gather_tile(g):
                er = pr.tile([128, E], FP16, name=f"er{g}", tag="er", bufs=3)
                nc.gpsimd.indirect_dma_start(
                    out=er[:], out_offset=None,
                    in_=emb_d[:],
                    in_offset=bass.IndirectOffsetOnAxis(ap=idx[:, g:g + 1], axis=0),
                )
                eT = pr.tile([128, KE, 128], FP16, name=f"eT{g}", tag="eT", bufs=8)
                nc.sync.dma_start_transpose(out=eT[:], in_=er[:])
                return eT

            eT_w = {g: gather_tile(g) for g in range(min(8, (n_steps + 1) // 2))}

            w_rz_g = [pc.tile([128, KH, 128], FP16, name=f"w_rz{g}")
                      for g in range(16)]
            w_hn_g = [pc.tile([128, KH, 128], FP16, name=f"w_hn{g}")
                      for g in range(8)]
            w_irz_g = [pc.tile([128, KE, 128], FP16, name=f"w_irz{g}")
                       for g in range(16)]
            w_in_g = [pc.tile([128, KE, 128], FP16, name=f"w_in{g}")
                      for g in range(8)]
            for c in range(8):
                for t, d, g in [(w_rz_g[c], w_rz_d, c),
                                (w_irz_g[c], w_irz_d, c),
                                (w_rz_g[8 + c], w_rz_d, 8 + c),
                                (w_irz_g[8 + c], w_irz_d, 8 + c),
                                (w_hn_g[c], w_hn_d, c),
                                (w_in_g[c], w_in_d, c)]:
                    nc.sync.dma_start(out=t[:], in_=d[g])
            w_out_n = [pc.tile([128, KH, NW], FP16, name=f"w_out{n}")
                       for n in range(NN)]
            nc.sync.dma_start(out=w_out_n[0][:], in_=w_outT_d[0])
            nc.sync.dma_start(out=b_out[:], in_=b_out_d[:])
            for n in range(1, NN):
                nc.sync.dma_start(out=w_out_n[n][:], in_=w_outT_d[n])

            hs_w = {}     # step-pair g -> [128, KH, 128] fp16

            # output jobs: (g, nn), 8 matmuls each, emitted 4/step from s=3.
            # The bias-add + store are deferred to end-of-step so they queue on
            # DVE/Pool after the chain ops instead of head-of-line blocking them.
            jobs = [(g, nn) for g in range(n_steps // 2) for nn in range(NN)]
            if not with_jobs:
                jobs = []
            jp = 0
            pending_adds = []

            def emit_job(g, nn):
                ps_o = pp.tile([128, NW], F32, name=f"pso{g}_{nn}", tag="pso",
                               bufs=4)
                hst = hs_w[g]
                for k in range(KH):
                    nc.tensor.matmul(
                        out=ps_o[:], lhsT=hst[:, k, :],
                        rhs=w_out_n[nn][:, k, :],
                        start=(k == 0), stop=(k == KH - 1),
                        skip_group_check=True)

                def add_and_store():
                    ob = pr.tile([128, NW], F32, name=f"ob{g}_{nn}", tag="ob",
                                 bufs=4)
                    nc.vector.tensor_tensor(
                        out=ob[:], in0=ps_o[:],
                        in1=b_out[:, nn * NW:(nn + 1) * NW], op=ADD)
                    nc.sync.dma_start(
                        out=out_d[g * 128:(g + 1) * 128, nn * NW:(nn + 1) * NW],
                        in_=ob[:])
                pending_adds.append(add_and_store)

            def job_ready(job, s):
                return 2 * job[0] + 2 <= s

            for s in range(n_steps):
                g, half = s // 2, s % 2
                eT = eT_w[g]

                if s == 0:
                    hprev = h0
                    hoff = 0
                else:
                    hprev = hs_w[(s - 1) // 2]
                    hoff = 64 * ((s - 1) % 2)
                if half == 0:
                    hs_w[g] = pr.tile([128, KH, 128], FP16, name=f"hs{g}",
                                      tag="hs", bufs=4)
                hcur = hs_w[g]
                hcoff = 64 * half

                # ---- output jobs first: they fill the PE while the previous
                # step's gate chains drain, so the rec below never stalls.
                if s >= 3:
                    for _ in range(4):
                        if jp < len(jobs) and job_ready(jobs[jp], s):
                            emit_job(*jobs[jp])
                            jp += 1

                # PSUM accumulation-group hazard: a start=True marks its whole
                # bank pending-zero, wiping any open partial sums there. So we
                # zero the rec psum banks with DVE memsets and accumulate with
                # start=False everywhere -- which makes ANY matmul interleaving
                # legal, letting the e-part (no h16 dependency) run first while
                # the previous step's gate chain drains.
                pss = []
                for j in range(4):
                    pj = pp.tile([128, 2, 4, 64], F32, name=f"ps{s}_{j}",
                                 tag="psc", bufs=4)
                    nc.vector.memset(pj[:], 0.0)
                    pss.append(pj)

                ps_l, r_l, z_l, t2_l = [], [], [], []
                for c in range(KH):
                    ps = pss[c // 2][:, c % 2]
                    for k in range(KE):
                        nc.tensor.matmul(
                            out=ps[:, 3, :], lhsT=w_in_g[c][:, k, :],
                            rhs=eT[:, k, 64 * half:64 * half + 64],
                            start=False, stop=(k == KE - 1),
                            skip_group_check=True)
                    for i, gc in ((0, c), (1, 8 + c)):
                        for k in range(KE):
                            nc.tensor.matmul(
                                out=ps[:, i, :], lhsT=w_irz_g[gc][:, k, :],
                                rhs=eT[:, k, 64 * half:64 * half + 64],
                                start=False, stop=False,
                                skip_group_check=True)
                for c in range(KH):
                    ps = pss[c // 2][:, c % 2]
                    for i, gc in ((0, c), (1, 8 + c)):
                        for k in range(KH):
                            nc.tensor.matmul(
                                out=ps[:, i, :], lhsT=w_rz_g[gc][:, k, :],
                                rhs=hprev[:, k, hoff:hoff + 64],
                                start=False, stop=(k == KH - 1),
                                skip_group_check=True)
                    for k in range(KH):
                        nc.tensor.matmul(
                            out=ps[:, 2, :], lhsT=w_hn_g[c][:, k, :],
                            rhs=hprev[:, k, hoff:hoff + 64],
                            start=False, stop=(k == KH - 1),
                            skip_group_check=True)

                    # ---- pass 1 of the gate chain: r/z sigmoids (ACT runs all
                    # 16 before any tanh, so it never head-of-line blocks) and
                    # the fused t/t2 ops. Everything reading PSUM must be on
                    # DVE or ACT: GPSIMD/Pool cannot access PSUM.
                    eng = nc.vector
                    r_c = pr.tile([128, 64], F32, name=f"r{s}_{c}", tag="r", bufs=3)
                    nc.scalar.activation(out=r_c[:], in_=ps[:, 0, :], func=SIG,
                                         bias=b_r[:, c:c + 1])
                    z_c = pr.tile([128, 64], F32, name=f"z{s}_{c}", tag="z",
                                  bufs=10)
                    nc.scalar.activation(out=z_c[:], in_=ps[:, 1, :],
                                         func=SIG, bias=b_z[:, c:c + 1])
                    t_c = pr.tile([128, 64], F32, name=f"t{s}_{c}", tag="t", bufs=3)
                    eng.scalar_tensor_tensor(
                        out=t_c[:], in0=ps[:, 2, :], scalar=b_hn[:, c:c + 1],
                        in1=r_c[:], op0=ADD, op1=MULT)
                    t2_c = pr.tile([128, 64], F32, name=f"t2{s}_{c}", tag="t2",
                                   bufs=10)
                    eng.scalar_tensor_tensor(
                        out=t2_c[:], in0=ps[:, 3, :], scalar=b_in[:, c:c + 1],
                        in1=t_c[:], op0=ADD, op1=ADD)
                    ps_l.append(ps); r_l.append(r_c); z_l.append(z_c)
                    t2_l.append(t2_c)

                # ---- pass 2: tanh + output-side ops per chunk, all on Pool
                # (SBUF-only inputs), keeping DVE free for psum work
                for c in range(KH):
                    eng = nc.gpsimd
                    z_c, t2_c = z_l[c], t2_l[c]
                    n_c = pr.tile([128, 64], F32, name=f"n{s}_{c}", tag="n", bufs=3)
                    nc.scalar.activation(out=n_c[:], in_=t2_c[:], func=TANH)
                    w_c = pr.tile([128, 64], F32, name=f"w{s}_{c}", tag="w", bufs=3)
                    eng.tensor_scalar(out=w_c[:], in0=z_c[:], scalar1=-1.0,
                                      scalar2=1.0, op0=MULT, op1=ADD)
                    u_c = pr.tile([128, 64], F32, name=f"u{s}_{c}", tag="u", bufs=3)
                    eng.tensor_tensor(out=u_c[:], in0=z_c[:],
                                      in1=hprev[:, c, hoff:hoff + 64], op=MULT)
                    q_c = pr.tile([128, 64], F32, name=f"q{s}_{c}", tag="q", bufs=3)
                    eng.tensor_tensor(out=q_c[:], in0=w_c[:], in1=n_c[:], op=MULT)
                    eng.tensor_tensor(
                        out=hcur[:, c, hcoff:hcoff + 64],
                        in0=q_c[:], in1=u_c[:], op=ADD)

                # ---- deferred job bias-adds: queued on DVE after this step's
                # chain ops so they never head-of-line block the chain
                for fn in pending_adds:
                    fn()
                pending_adds.clear()

                # ---- prefetch next embedding tile
                if half == 1 and g + 8 < (n_steps + 1) // 2:
                    eT_w[g + 8] = gather_tile(g + 8)

            # ---- drain remaining output jobs
            while jp < len(jobs):
                emit_job(*jobs[jp])
                jp += 1
            for fn in pending_adds:
                fn()
            pending_adds.clear()

    nc.compile()
    _CACHE[key] = nc
    return nc


def _prep_in_maps(x, hidden, emb, w_ih, w_hh, b_ih, b_hh, w_out, b_out):
    f16, f32 = np.float16, np.float32

    toks = np.concatenate([np.full((1, B), 2, dtype=np.int64),
                           np.asarray(x)[:-1].astype(np.int64)], axis=0)
    t_flat = toks.reshape(SB).astype(np.int32)
    idx = np.ascontiguousarray(t_flat.reshape(NT, 128).T)        # [128, 32]

    emb_t = np.asarray(emb, dtype=f32).copy()
    emb_t[0] = 0.0
    emb_t = np.maximum(emb_t, 0.0).astype(f16)                    # relu folded

    w_hh = np.asarray(w_hh, dtype=f32)
    w_ih = np.asarray(w_ih, dtype=f32)

    def gview(m, kc, gc):  # [rows, K] -> [gc, 128, kc, 128] gates-major lhsT
        return np.ascontiguousarray(
            m.reshape(gc, 128, kc, 128).transpose(0, 3, 2, 1)).astype(f16)

    w_rz = gview(w_hh[0:2 * H], KH, 16)
    w_hn = gview(w_hh[2 * H:3 * H], KH, 8)
    w_irz = gview(w_ih[0:2 * H], KE, 16)
    w_in = gview(w_ih[2 * H:3 * H], KE, 8)

    b_ih = np.asarray(b_ih, dtype=f32)
    b_hh = np.asarray(b_hh, dtype=f32)

    def bview(v):  # [H] -> [128, 8]: col c, part p = v[128c + p]
        return np.ascontiguousarray(v.reshape(8, 128).T).astype(f32)

    b_r = bview(b_ih[0:H] + b_hh[0:H])
    b_z = bview(b_ih[H:2 * H] + b_hh[H:2 * H])
    b_hn_t = bview(b_hh[2 * H:3 * H])
    b_in_t = bview(b_ih[2 * H:3 * H])

    h0 = np.asarray(hidden, dtype=f32)[0]                         # [B, H]
    h0_gm = np.ascontiguousarray(
        h0.T.reshape(8, 128, B).transpose(1, 0, 2)).astype(f16)   # [128, 8, 64]

    w_out = np.asarray(w_out, dtype=f32)
    b_out = np.asarray(b_out, dtype=f32)
    NV = (VC + 127) // 128

    shared = dict(
        emb_t=emb_t, idx=idx,
        w_rz=w_rz, w_hn=w_hn, w_irz=w_irz, w_in=w_in,
        b_r=b_r, b_z=b_z, b_hn=b_hn_t, b_in=b_in_t,
        h0=h0_gm,
    )
    in_maps = []
    for c in range(NCORES):
        sl = slice(c * VC, (c + 1) * VC)
        w_outT = np.ascontiguousarray(
            w_out[sl].T.reshape(KH, 128, NN, NW).transpose(2, 1, 0, 3)).astype(f16)
        b_out_bc = np.ascontiguousarray(
            np.broadcast_to(b_out[sl], (128, VC))).astype(f16)
        in_maps.append(dict(shared, w_outT=w_outT, b_out_bc=b_out_bc))
    return in_maps


def _assemble(results):
    full = np.concatenate(
        [r["out"].reshape(S, B, VC) for r in results], axis=2)   # (S, B, V)
    return np.ascontiguousarray(full.transpose(1, 0, 2)[None]).astype(np.float32)


def _run(trace=False, tmpdir=None, **inputs):
    nc = _build()
    in_maps = _prep_in_maps(**inputs)
    res = run_bass_kernel_spmd(nc, in_maps, list(range(NCORES)),
                               trace=trace, tmpdir=tmpdir)
    return _assemble(res.results), res


def kernel(**inputs) -> np.ndarray:
    out, _ = _run(**inputs)
    return out


if __name__ == "__main__":
    rng = np.random.default_rng(0)
    ins = dict(
        x=rng.integers(0, V, (S, B)).astype(np.int32),
        hidden=rng.standard_normal((1, B, H)).astype(np.float32),
        emb=rng.standard_normal((V, E)).astype(np.float32),
        w_ih=rng.uniform(-1 / 32, 1 / 32, (3 * H, E)).astype(np.float32),
        w_hh=rng.uniform(-1 / 32, 1 / 32, (3 * H, H)).astype(np.float32),
        b_ih=rng.uniform(-1 / 32, 1 / 32, (3 * H,)).astype(np.float32),
        b_hh=rng.uniform(-1 / 32, 1 / 32, (3 * H,)).astype(np.float32),
        w_out=rng.uniform(-1 / 32, 1 / 32, (V, H)).astype(np.float32),
        b_out=rng.uniform(-1 / 32, 1 / 32, (V,)).astype(np.float32),
    )
    out = kernel(**ins)
    print("out", out.shape, out.dtype, float(np.abs(out).max()))


# revision 3
# speedup vs baseline: 1.0017x; 1.0017x over previous
"""DecoderRNN (GRU + embedding + vocab projection) Bass kernel, v2.

Gates-major recurrence: gate rows live on PSUM partitions (24 chunks of
128 across r/z/n), batch=64 is the streamed free dim. Each matmul runs
the full 128-wide PE array (vs. 64-wide in the batch-major layout), so
the recurrence costs 18.4k PE cycles/step instead of 36.9k — and h'
comes out of the gate chain already in the [H-on-partitions, batch]
layout the next step's matmuls (and the output projection's lhsT) need,
so the per-step transpose matmul disappears entirely.

  toks = [2, x[0..S-2]]                  (teacher forcing, S=64, B=64)
  e_s  = relu(emb[toks_s])               (E=512, padding row 0 = 0)
  r/z  = sig(W e + U h + b), n = tanh(gin + r*ghn),  h' = n + z*(h-n)
  logits_s = h_s @ Wout^T + b_out        (V=32000)

Distribution: recurrence redundant on all 8 cores (SPMD single-program;
per-step cross-core gathers cost more than they save), output projection
vocab-split 8 ways (4000 cols/core), its matmuls interleaved into the
per-step gate-chain windows to keep the PE busy.

Layouts (gates-major):
  hs tile [128, 8, 128] fp16 per step-pair g: hs[p, kc, 64*half+b]
    = h_{2g+half}[b, 128*kc+p] — rhs for the recurrence (stream batch),
    lhsT for the output projection (128-token stationary).
  psum ps_rz [128, 16, 64]: chunk c = gate r rows 128c..', chunk 8+c = z.
  ps_hn/ps_gin [128, 8, 64]: n-gate h-part / e-part (kept separate).
  w_* [128, kc, gc, 128] fp16: w[p, kc, gc, m] = W[128*gc+m, 128*kc+p].
Matmuls fp16 (fp32 PSUM accumulate); gate arithmetic fp32 with biases
fused into ACT sigmoid / Pool tensor_scalar; h carried in fp16 only.
"""

import sys

sys.path.insert(0, "/opt/trn_rl_repo")

import numpy as np

import concourse.bass as bass
import concourse.bacc as bacc
import concourse.mybir as mybir
import concourse.tile as tile
from concourse.bass_utils import run_bass_kernel_spmd

FP16 = mybir.dt.float16
F32 = mybir.dt.float32
I32 = mybir.dt.int32
SIG = mybir.ActivationFunctionType.Sigmoid
TANH = mybir.ActivationFunctionType.Tanh
ADD = mybir.AluOpType.add
SUB = mybir.AluOpType.subtract
MULT = mybir.AluOpType.mult

S, B, H, E, V = 64, 64, 1024, 512, 32000
NCORES = 8
VC = V // NCORES          # 4000 vocab cols per core
SB = S * B                # 4096
KH = H // 128             # 8 h k-chunks
KE = E // 128             # 4 e k-chunks
NN = 8                    # output n-chunks per core
NW = VC // NN             # 500 cols per n-chunk
NT = SB // 128            # 32 output row tiles (= step pairs)

_CACHE = {}


def _build(n_steps=S, with_jobs=True):
    key = ("nc2", n_steps, with_jobs)
    if key in _CACHE:
        return _CACHE[key]

    nc = bacc.Bacc("TRN2", target_bir_lowering=False, debug=False)

    def din(name, shape, dt):
        return nc.dram_tensor(name, shape, dt, kind="ExternalInput").ap()

    emb_d = din("emb_t", [V, E], FP16)
    idx_d = din("idx", [128, NT], I32)
    w_rz_d = din("w_rz", [16, 128, KH, 128], FP16)
    w_hn_d = din("w_hn", [8, 128, KH, 128], FP16)
    w_irz_d = din("w_irz", [16, 128, KE, 128], FP16)
    w_in_d = din("w_in", [8, 128, KE, 128], FP16)
    b_r_d = din("b_r", [128, 8], F32)
    b_z_d = din("b_z", [128, 8], F32)
    b_hn_d = din("b_hn", [128, 8], F32)
    b_in_d = din("b_in", [128, 8], F32)
    h0_d = din("h0", [128, 8, 64], FP16)
    w_outT_d = din("w_outT", [NN, 128, KH, NW], FP16)
    b_out_d = din("b_out_bc", [128, VC], FP16)
    out_d = nc.dram_tensor("out", [SB, VC], F32, kind="ExternalOutput").ap()

    with tile.TileContext(nc) as tc:
        with tc.tile_pool(name="const", bufs=1) as pc, \
             tc.tile_pool(name="roll", bufs=1) as pr, \
             tc.tile_pool(name="psum", bufs=1, space="PSUM") as pp:

            # ---- constants in SBUF, DMA'd in consumption order so step 0
            # starts after ~1 MB instead of the full 18 MB of weights.
            b_r = pc.tile([128, 8], F32, name="b_r")
            b_z = pc.tile([128, 8], F32, name="b_z")
            b_hn = pc.tile([128, 8], F32, name="b_hn")
            b_in = pc.tile([128, 8], F32, name="b_in")
            h0 = pc.tile([128, 8, 64], FP16, name="h0")
            b_out = pc.tile([128, VC], FP16, name="b_out")
            idx = pc.tile([128, NT], I32, name="idx")
            for t, d in [(idx, idx_d), (h0, h0_d), (b_r, b_r_d), (b_z, b_z_d),
                         (b_hn, b_hn_d), (b_in, b_in_d)]:
                nc.sync.dma_start(out=t[:], in_=d[:])

            # ---- embedding gather pipeline (tile g = tokens of steps 2g, 2g+1)
            # primed BEFORE the weight DMAs: the transposes ride the SP HWDGE
            # queue, and behind 17.6 MB of weights the first eT would otherwise
            # arrive at ~66us instead of ~5us.
            def gather_tile(g):
                er = pr.tile([128, E], FP16, name=f"er{g}", tag="er", bufs=3)
                nc.gpsimd.indirect_dma_start(
                    out=er[:], out_offset=None,
                    in_=emb_d[:],
                    in_offset=bass.IndirectOffsetOnAxis(ap=idx[:, g:g + 1], axis=0),
                )
                eT = pr.tile([128, KE, 128], FP16, name=f"eT{g}", tag="eT", bufs=8)
                nc.sync.dma_start_transpose(out=eT[:], in_=er[:])
                return eT

            eT_w = {g: gather_tile(g) for g in range(min(8, (n_steps + 1) // 2))}

            w_rz_g = [pc.tile([128, KH, 128], FP16, name=f"w_rz{g}")
                      for g in range(16)]
            w_hn_g = [pc.tile([128, KH, 128], FP16, name=f"w_hn{g}")
                      for g in range(8)]
            w_irz_g = [pc.tile([128, KE, 128], FP16, name=f"w_irz{g}")
                       for g in range(16)]
            w_in_g = [pc.tile([128, KE, 128], FP16, name=f"w_in{g}")
                      for g in range(8)]
            for c in range(8):
                for t, d, g in [(w_rz_g[c], w_rz_d, c),
                                (w_irz_g[c], w_irz_d, c),
                                (w_rz_g[8 + c], w_rz_d, 8 + c),
                                (w_irz_g[8 + c], w_irz_d, 8 + c),
                                (w_hn_g[c], w_hn_d, c),
                                (w_in_g[c], w_in_d, c)]:
                    nc.sync.dma_start(out=t[:], in_=d[g])
            w_out_n = [pc.tile([128, KH, NW], FP16, name=f"w_out{n}")
                       for n in range(NN)]
            nc.sync.dma_start(out=w_out_n[0][:], in_=w_outT_d[0])
            nc.sync.dma_start(out=b_out[:], in_=b_out_d[:])
            for n in range(1, NN):
                nc.sync.dma_start(out=w_out_n[n][:], in_=w_outT_d[n])

            hs_w = {}     # step-pair g -> [128, KH, 128] fp16

            # output jobs: (g, nn), 8 matmuls each, emitted 4/step from s=3.
            # The bias-add + store are deferred to end-of-step so they queue on
            # DVE/Pool after the chain ops instead of head-of-line blocking them.
            jobs = [(g, nn) for g in range(n_steps // 2) for nn in range(NN)]
            if not with_jobs:
                jobs = []
            jp = 0
            pending_adds = []

            def emit_job(g, nn):
                ps_o = pp.tile([128, NW], F32, name=f"pso{g}_{nn}", tag="pso",
                               bufs=3)
                hst = hs_w[g]
                for k in range(KH):
                    nc.tensor.matmul(
                        out=ps_o[:], lhsT=hst[:, k, :],
                        rhs=w_out_n[nn][:, k, :],
                        start=(k == 0), stop=(k == KH - 1),
                        skip_group_check=True)

                def add_and_store():
                    ob = pr.tile([128, NW], F32, name=f"ob{g}_{nn}", tag="ob",
                                 bufs=8)
                    nc.vector.tensor_tensor(
                        out=ob[:], in0=ps_o[:],
                        in1=b_out[:, nn * NW:(nn + 1) * NW], op=ADD)
                    nc.sync.dma_start(
                        out=out_d[g * 128:(g + 1) * 128, nn * NW:(nn + 1) * NW],
                        in_=ob[:])
                pending_adds.append(add_and_store)

            def job_ready(job, s):
                # +3: consume tiles only once they are >=1 step old, so the
                # step-boundary wait for the newest tile's last h16 disappears
                return 2 * job[0] + 3 <= s

            for s in range(n_steps):
                g, half = s // 2, s % 2
                eT = eT_w[g]

                if s == 0:
                    hprev = h0
                    hoff = 0
                else:
                    hprev = hs_w[(s - 1) // 2]
                    hoff = 64 * ((s - 1) % 2)
                if half == 0:
                    hs_w[g] = pr.tile([128, KH, 128], FP16, name=f"hs{g}",
                                      tag="hs", bufs=4)
                hcur = hs_w[g]
                hcoff = 64 * half

                # ---- output jobs first: they fill the PE while the previous
                # step's gate chains drain, so the rec below never stalls.
                # Their bias-adds flush immediately after on DVE (it is idle
                # until the chain's t/t2 ops ~10us later), freeing psum slots
                # fast enough for a 3-deep pso rotation.
                if s >= 3:
                    for _ in range(4):
                        if jp < len(jobs) and job_ready(jobs[jp], s):
                            emit_job(*jobs[jp])
                            jp += 1
                    for fn in pending_adds:
                        fn()
                    pending_adds.clear()

                # ---- prefetch the next embedding tile at the TOP of the step:
                # the ~1us SWDGE descriptor generation runs on the Pool queue,
                # which is idle here but busy with chain-tail ops at step end.
                if s % 2 == 0 and s >= 2 and s // 2 + 7 < (n_steps + 1) // 2:
                    eT_w[s // 2 + 7] = gather_tile(s // 2 + 7)

                # PSUM start=True marks the instruction's whole bank pending-
                # zero: every byte's NEXT write overwrites instead of
                # accumulating. We exploit that deliberately -- the single
                # opening gin matmul of each bank carries start=True, zeroing
                # the bank for all six groups sharing it, and every other
                # matmul accumulates with start=False. Any interleaving is then
                # legal, letting the whole e-part (no h16 dependency) run first
                # while the previous step's gate chain drains.
                pss = []
                for j in range(4):
                    pj = pp.tile([128, 2, 4, 64], F32, name=f"ps{s}_{j}",
                                 tag="psc", bufs=5)
                    pss.append(pj)

                ps_l, r_l, z_l, t2_l = [], [], [], []
                for c in range(KH):
                    ps = pss[c // 2][:, c % 2]
                    for k in range(KE):
                        nc.tensor.matmul(
                            out=ps[:, 3, :], lhsT=w_in_g[c][:, k, :],
                            rhs=eT[:, k, 64 * half:64 * half + 64],
                            start=(k == 0 and c % 2 == 0), stop=(k == KE - 1),
                            skip_group_check=True)
                    for i, gc in ((0, c), (1, 8 + c)):
                        for k in range(KE):
                            nc.tensor.matmul(
                                out=ps[:, i, :], lhsT=w_irz_g[gc][:, k, :],
                                rhs=eT[:, k, 64 * half:64 * half + 64],
                                start=False, stop=False,
                                skip_group_check=True)
                def emit_tail(c):
                    # chain tail for chunk c: tanh + the Pool-side output ops.
                    # Runs with a 2-chunk lookahead from the pass-1 loop, so
                    # ACT never head-of-line blocks (t2(c) is ready before the
                    # ACT queue reaches tanh(c)) while h16 chunks still land
                    # DURING the rec instead of all after it.
                    eng = nc.gpsimd
                    z_c, t2_c = z_l[c], t2_l[c]
                    n_c = pr.tile([128, 64], F32, name=f"n{s}_{c}", tag="n",
                                  bufs=3)
                    nc.scalar.activation(out=n_c[:], in_=t2_c[:], func=TANH)
                    w_c = pr.tile([128, 64], F32, name=f"w{s}_{c}", tag="w",
                                  bufs=3)
                    eng.tensor_scalar(out=w_c[:], in0=z_c[:], scalar1=-1.0,
                                      scalar2=1.0, op0=MULT, op1=ADD)
                    u_c = pr.tile([128, 64], F32, name=f"u{s}_{c}", tag="u",
                                  bufs=3)
                    eng.tensor_tensor(out=u_c[:], in0=z_c[:],
                                      in1=hprev[:, c, hoff:hoff + 64], op=MULT)
                    q_c = pr.tile([128, 64], F32, name=f"q{s}_{c}", tag="q",
                                  bufs=3)
                    eng.tensor_tensor(out=q_c[:], in0=w_c[:], in1=n_c[:], op=MULT)
                    eng.tensor_tensor(
                        out=hcur[:, c, hcoff:hcoff + 64],
                        in0=q_c[:], in1=u_c[:], op=ADD)

                for c in range(KH):
                    ps = pss[c // 2][:, c % 2]
                    for i, gc in ((0, c), (1, 8 + c)):
                        for k in range(KH):
                            nc.tensor.matmul(
                                out=ps[:, i, :], lhsT=w_rz_g[gc][:, k, :],
                                rhs=hprev[:, k, hoff:hoff + 64],
                                start=False, stop=(k == KH - 1),
                                skip_group_check=True)
                    for k in range(KH):
                        nc.tensor.matmul(
                            out=ps[:, 2, :], lhsT=w_hn_g[c][:, k, :],
                            rhs=hprev[:, k, hoff:hoff + 64],
                            start=False, stop=(k == KH - 1),
                            skip_group_check=True)

                    # ---- pass 1 of the gate chain: r/z sigmoids (ACT runs all
                    # 16 before any tanh, so it never head-of-line blocks) and
                    # the fused t/t2 ops. Everything reading PSUM must be on
                    # DVE or ACT: GPSIMD/Pool cannot access PSUM.
                    eng = nc.vector
                    r_c = pr.tile([128, 64], F32, name=f"r{s}_{c}", tag="r", bufs=3)
                    nc.scalar.activation(out=r_c[:], in_=ps[:, 0, :], func=SIG,
                                         bias=b_r[:, c:c + 1])
                    z_c = pr.tile([128, 64], F32, name=f"z{s}_{c}", tag="z",
                                  bufs=10)
                    nc.scalar.activation(out=z_c[:], in_=ps[:, 1, :],
                                         func=SIG, bias=b_z[:, c:c + 1])
                    t_c = pr.tile([128, 64], F32, name=f"t{s}_{c}", tag="t", bufs=3)
                    eng.scalar_tensor_tensor(
                        out=t_c[:], in0=ps[:, 2, :], scalar=b_hn[:, c:c + 1],
                        in1=r_c[:], op0=ADD, op1=MULT)
                    t2_c = pr.tile([128, 64], F32, name=f"t2{s}_{c}", tag="t2",
                                   bufs=10)
                    eng.scalar_tensor_tensor(
                        out=t2_c[:], in0=ps[:, 3, :], scalar=b_in[:, c:c + 1],
                        in1=t_c[:], op0=ADD, op1=ADD)
                    ps_l.append(ps); r_l.append(r_c); z_l.append(z_c)
                    t2_l.append(t2_c)

                    # chain tail with 2-chunk lookahead
                    if c >= 2:
                        emit_tail(c - 2)

                emit_tail(KH - 2)
                emit_tail(KH - 1)
                for fn in pending_adds:
                    fn()
                pending_adds.clear()

            # ---- drain remaining output jobs
            while jp < len(jobs):
                emit_job(*jobs[jp])
                jp += 1
            for fn in pending_adds:
                fn()
            pending_adds.clear()

    nc.compile()
    _CACHE[key] = nc
    return nc


def _prep_in_maps(x, hidden, emb, w_ih, w_hh, b_ih, b_hh, w_out, b_out):
    f16, f32 = np.float16, np.float32

    toks = np.concatenate([np.full((1, B), 2, dtype=np.int64),
                           np.asarray(x)[:-1].astype(np.int64)], axis=0)
    t_flat = toks.reshape(SB).astype(np.int32)
    idx = np.ascontiguousarray(t_flat.reshape(NT, 128).T)        # [128, 32]

    emb_t = np.asarray(emb, dtype=f32).copy()
    emb_t[0] = 0.0
    emb_t = np.maximum(emb_t, 0.0).astype(f16)                    # relu folded

    w_hh = np.asarray(w_hh, dtype=f32)
    w_ih = np.asarray(w_ih, dtype=f32)

    def gview(m, kc, gc):  # [rows, K] -> [gc, 128, kc, 128] gates-major lhsT
        return np.ascontiguousarray(
            m.reshape(gc, 128, kc, 128).transpose(0, 3, 2, 1)).astype(f16)

    w_rz = gview(w_hh[0:2 * H], KH, 16)
    w_hn = gview(w_hh[2 * H:3 * H], KH, 8)
    w_irz = gview(w_ih[0:2 * H], KE, 16)
    w_in = gview(w_ih[2 * H:3 * H], KE, 8)

    b_ih = np.asarray(b_ih, dtype=f32)
    b_hh = np.asarray(b_hh, dtype=f32)

    def bview(v):  # [H] -> [128, 8]: col c, part p = v[128c + p]
        return np.ascontiguousarray(v.reshape(8, 128).T).astype(f32)

    b_r = bview(b_ih[0:H] + b_hh[0:H])
    b_z = bview(b_ih[H:2 * H] + b_hh[H:2 * H])
    b_hn_t = bview(b_hh[2 * H:3 * H])
    b_in_t = bview(b_ih[2 * H:3 * H])

    h0 = np.asarray(hidden, dtype=f32)[0]                         # [B, H]
    h0_gm = np.ascontiguousarray(
        h0.T.reshape(8, 128, B).transpose(1, 0, 2)).astype(f16)   # [128, 8, 64]

    w_out = np.asarray(w_out, dtype=f32)
    b_out = np.asarray(b_out, dtype=f32)
    NV = (VC + 127) // 128

    shared = dict(
        emb_t=emb_t, idx=idx,
        w_rz=w_rz, w_hn=w_hn, w_irz=w_irz, w_in=w_in,
        b_r=b_r, b_z=b_z, b_hn=b_hn_t, b_in=b_in_t,
        h0=h0_gm,
    )
    in_maps = []
    for c in range(NCORES):
        sl = slice(c * VC, (c + 1) * VC)
        w_outT = np.ascontiguousarray(
            w_out[sl].T.reshape(KH, 128, NN, NW).transpose(2, 1, 0, 3)).astype(f16)
        b_out_bc = np.ascontiguousarray(
            np.broadcast_to(b_out[sl], (128, VC))).astype(f16)
        in_maps.append(dict(shared, w_outT=w_outT, b_out_bc=b_out_bc))
    return in_maps


def _assemble(results):
    full = np.concatenate(
        [r["out"].reshape(S, B, VC) for r in results], axis=2)   # (S, B, V)
    return np.ascontiguousarray(full.transpose(1, 0, 2)[None]).astype(np.float32)


def _run(trace=False, tmpdir=None, **inputs):
    nc = _build()
    in_maps = _prep_in_maps(**inputs)
    res = run_bass_kernel_spmd(nc, in_maps, list(range(NCORES)),
                               trace=trace, tmpdir=tmpdir)
    return _assemble(res.results), res


def kernel(**inputs) -> np.ndarray:
    out, _ = _run(**inputs)
    return out


if __name__ == "__main__":
    rng = np.random.default_rng(0)
    ins = dict(
        x=rng.integers(0, V, (S, B)).astype(np.int32),
        hidden=rng.standard_normal((1, B, H)).astype(np.float32),
        emb=rng.standard_normal((V, E)).astype(np.float32),
        w_ih=rng.uniform(-1 / 32, 1 / 32, (3 * H, E)).astype(np.float32),
        w_hh=rng.uniform(-1 / 32, 1 / 32, (3 * H, H)).astype(np.float32),
        b_ih=rng.uniform(-1 / 32, 1 / 32, (3 * H,)).astype(np.float32),
        b_hh=rng.uniform(-1 / 32, 1 / 32, (3 * H,)).astype(np.float32),
        w_out=rng.uniform(-1 / 32, 1 / 32, (V, H)).astype(np.float32),
        b_out=rng.uniform(-1 / 32, 1 / 32, (V,)).astype(np.float32),
    )
    out = kernel(**ins)
    print("out", out.shape, out.dtype, float(np.abs(out).max()))
